# revision 23
# baseline (speedup 1.0000x reference)
"""ExpertScatter TRN2 kernel.

reference semantics:
    X = einsum('bekj,eji->beki', Y, W)          # per-head projection
    out[b] = zeros([T, I]); out[b, Ind[b,e,k]] += X[b,e,k]

Strategy (data-parallel over batch, 1 batch per NeuronCore):
  Host pre-aggregates, per (batch, head), the Y rows that share a target
  slot (segment-sum over slot-sorted rows — free on host, and exact in
  float64).  Per head that leaves ~906 distinct-slot rows instead of 1024.
  The device then only has to
    Phase A: project the aggregated rows: X_chunk[128, 1024] =
             Yt_chunk.T @ W[e] (fp16 operands, fp32 PSUM), copy PSUM->SBUF
             fp16 (alternating full-width copies on DVE / Activation so
             neither engine becomes the bottleneck), and
    Phase B: dma_scatter_add the SBUF rows straight into the HBM output at
             their slot addresses (out[idx] += row).  No X round-trip
             through HBM, no gather, no one-hot matmuls.  The runtime
             hands the kernel a zero-initialized ExternalOutput buffer, so
             the scatter-add base is well-defined.

  The scatter's index table layout ("wrapped in 16 partitions") and the
  SBUF source layout (row i lives in partition i%128, free-slot i//128)
  exactly match the natural matmul-chunk layout, so no on-chip reshuffle
  is needed.  Per-head index counts are padded to a static multiple of 16
  (max over the 8 cores) with a trash slot (row T_SLOTS of the output,
  stripped on host); the padded Y columns are zero so they contribute 0.
"""

import os

import numpy as np

import concourse.bacc as bacc
import concourse.mybir as mybir
import concourse.tile as tile
from concourse.bass_utils import run_bass_kernel_spmd

# Problem constants (hardcoded per harness contract).
B = 8
HEADS = 16
K = 1024
HEAD_DIM = 128
OUT_DIM = 1024
T_SLOTS = 4096

NCORES = 8

F32 = mybir.dt.float32
FP16 = mybir.dt.float16
I16 = mybir.dt.int16

PF = int(os.environ.get("ES_PF", "6"))          # heads prefetched ahead
XBUFS = int(os.environ.get("ES_XBUFS", "4"))
YBUFS = int(os.environ.get("ES_YBUFS", "10"))
WBUFS = int(os.environ.get("ES_WBUFS", "10"))
PABUFS = int(os.environ.get("ES_PABUFS", "4"))
# Chunks per scatter part: each head's scatter is split at chunk boundaries
# so the first part can fire before the whole head is copied (earlier DMA
# engagement, shorter tail drain).
SPLIT_CHUNKS = int(os.environ.get("ES_SPLIT_CHUNKS", "2"))
# Scatter-adds to one DRAM tensor get WAW-serialized by the tile framework
# (each waits on the previous one's DMA-completion sem, ~3us dead time per
# scatter).  Adds commute, so round-robin the heads over NCHAINS independent
# output tensors and sum them on the host; chains interleave on the DMA
# engines and hide the per-chain serialization.
NCHAINS = int(os.environ.get("ES_NCHAINS", "4"))

_cache = {}


def _build_program(ne_list):
    """ne_list: per-head static padded index counts (multiples of 16)."""
    nidx_cols = sum(n // 16 for n in ne_list)
    ycols = sum(ne_list)          # compact: only the real+pad16 columns
    yofs = [sum(ne_list[:e]) for e in range(HEADS)]

    nc = bacc.Bacc("TRN2", target_bir_lowering=False, debug=False,
                   num_devices=NCORES)

    yt = nc.dram_tensor("yt", [HEAD_DIM, ycols], FP16,
                        kind="ExternalInput").ap()
    w = nc.dram_tensor("w", [HEAD_DIM, HEADS * OUT_DIM], FP16,
                       kind="ExternalInput").ap()
    sidx = nc.dram_tensor("sidx", [128, nidx_cols], I16,
                          kind="ExternalInput").ap()
    outs = [nc.dram_tensor(f"out{q}", [T_SLOTS + 1, OUT_DIM], FP16,
                           kind="ExternalOutput").ap()
            for q in range(NCHAINS)]

    with tile.TileContext(nc) as tc:
        with (
            tc.tile_pool(name="const", bufs=1) as cpool,
            tc.tile_pool(name="yhead", bufs=YBUFS) as ypool,
            tc.tile_pool(name="whead", bufs=WBUFS) as wpool,
            tc.tile_pool(name="xtile", bufs=XBUFS) as xpool,
            tc.tile_pool(name="psumA", bufs=PABUFS, space="PSUM") as pspool,
        ):
            sidx_sb = cpool.tile([128, nidx_cols], I16, tag="sidx")

            yts, ws = {}, {}

            def load_head(e):
                ws[e] = wpool.tile([128, OUT_DIM], FP16, tag="w", name=f"w{e}")
                nc.sync.dma_start(out=ws[e][:],
                                  in_=w[:, e * OUT_DIM:(e + 1) * OUT_DIM])
                ne = ne_list[e]
                ncols = -(-ne // 128) * 128
                yts[e] = ypool.tile([128, ncols], FP16, tag="yt",
                                    name=f"yt{e}")
                nc.sync.dma_start(out=yts[e][:, :ne],
                                  in_=yt[:, yofs[e]:yofs[e] + ne])
                if ne < ncols:
                    # Zero the chunk-alignment tail so the last chunk's
                    # matmul never reads uninitialized SBUF.
                    nc.gpsimd.memset(yts[e][:, ne:], 0.0)

            load_head(0)
            for ee in range(1, 1 + PF):
                load_head(ee)
            # Index table after the prefetch burst: its small transfer should
            # not occupy an early DMA slot while the pipeline is ramping.
            nc.sync.dma_start(out=sidx_sb[:], in_=sidx[:])

            c0 = 0
            chain = 0
            for e in range(HEADS):
                yt_e = yts.pop(e)
                w_e = ws.pop(e)
                if e + PF + 1 < HEADS:
                    load_head(e + PF + 1)
                ne = ne_list[e]
                nchunks = -(-ne // 128)
                xe = xpool.tile([128, nchunks, OUT_DIM], FP16, tag="x",
                                name=f"x{e}")

                # Scatter-part boundaries at multiples of SPLIT_CHUNKS chunks
                # (source row i of a call reads partition i%128 of its in_ap,
                # so starts must be chunk-aligned; the tail takes the rest).
                bounds = list(range(0, nchunks, SPLIT_CHUNKS)) + [nchunks]
                part = 0
                for c in range(nchunks):
                    px = pspool.tile([128, OUT_DIM], F32, tag="pa")
                    lhsT = yt_e[:, c * 128:(c + 1) * 128]
                    for h in range(2):
                        nc.tensor.matmul(
                            out=px[:, h * 512:(h + 1) * 512],
                            lhsT=lhsT,
                            rhs=w_e[:, h * 512:(h + 1) * 512],
                            start=True, stop=True,
                        )
                    # Full-width copies, alternating engines: one PSUM-access
                    # bubble per 1024 cols instead of two.
                    if c % 2 == 0:
                        nc.vector.tensor_copy(out=xe[:, c, :], in_=px[:])
                    else:
                        nc.scalar.copy(out=xe[:, c, :], in_=px[:])
                    if c + 1 == bounds[part + 1]:
                        g0, g1 = bounds[part], bounds[part + 1]
                        r0 = g0 * 128
                        n = min(ne, g1 * 128) - r0
                        if n > 0:
                            nc.gpsimd.dma_scatter_add(
                                out_ap=outs[chain % NCHAINS][:],
                                in_ap=xe[:, g0:g1, :],
                                idxs_ap=sidx_sb[:, c0 + r0 // 16:
                                                c0 + (r0 + n) // 16],
                                num_idxs=n, num_idxs_reg=n,
                                elem_size=OUT_DIM,
                            )
                            chain += 1
                        part += 1
                c0 += ne // 16

    nc.compile()
    return nc


def _get_program(ne_list):
    key = (tuple(ne_list), PF, XBUFS, YBUFS, WBUFS, PABUFS, SPLIT_CHUNKS,
           NCHAINS)
    if key not in _cache:
        _cache[key] = _build_program(ne_list)
    return _cache[key]


def _prep_core_inputs(Yb, Indb, ne_list):
    """Host prep for one batch: per head, slot-sort + segment-sum Y rows,
    transpose into the compact yt, and build the wrapped scatter-index
    table."""
    yofs = [sum(ne_list[:e]) for e in range(HEADS)]
    yt = np.zeros((HEAD_DIM, sum(ne_list)), dtype=np.float32)
    idx_blocks = []
    for e in range(HEADS):
        ind = Indb[e].astype(np.int64)
        order = np.argsort(ind, kind="stable")
        s_sorted = ind[order]
        y_sorted = Yb[e][order].astype(np.float64)
        uniq, starts = np.unique(s_sorted, return_index=True)
        agg = np.add.reduceat(y_sorted, starts, axis=0)      # [D, 128]
        d = len(uniq)
        ne = ne_list[e]
        assert d <= ne, f"head {e}: {d} > padded {ne}"
        yt[:, yofs[e]:yofs[e] + d] = agg.T.astype(np.float32)
        col = np.full(ne, T_SLOTS, dtype=np.int16)
        col[:d] = uniq.astype(np.int16)
        idx_blocks.append(col.reshape(ne // 16, 16).T)       # [16, ne/16]
    blk = np.concatenate(idx_blocks, axis=1)
    sidx = np.ascontiguousarray(np.tile(blk, (8, 1)), dtype=np.int16)
    return yt, sidx


def kernel(Y, Ind, T, W):
    Y = np.asarray(Y, dtype=np.float32)
    Ind = np.asarray(Ind)
    W = np.asarray(W, dtype=np.float32)
    assert int(T) == T_SLOTS and Y.shape == (B, HEADS, K, HEAD_DIM)

    w_in = np.ascontiguousarray(
        W.transpose(1, 0, 2).reshape(HEAD_DIM, HEADS * OUT_DIM)
    ).astype(np.float16)

    # Static per-head padded counts: max distinct-slot count over the 8
    # cores, rounded up to 16 (scatter index-table granularity).
    d_counts = np.zeros((B, HEADS), dtype=np.int64)
    for b in range(B):
        for e in range(HEADS):
            d_counts[b, e] = np.unique(Ind[b, e]).size
    ne_list = [int(-(-int(d_counts[:, e].max()) // 16) * 16)
               for e in range(HEADS)]

    nc = _get_program(ne_list)

    in_maps = []
    for b in range(B):
        yt, sidx = _prep_core_inputs(Y[b], Ind[b], ne_list)
        in_maps.append({
            "yt": yt.astype(np.float16), "w": w_in, "sidx": sidx,
        })

    # The first execution of a freshly compiled NEFF occasionally wedges a
    # core (NRT_EXEC_UNIT_UNRECOVERABLE); a retry on a fresh execute has
    # been observed to recover.
    last_exc = None
    for attempt in range(3):
        try:
            res = run_bass_kernel_spmd(
                nc, in_maps, core_ids=list(range(NCORES)),
                trace=os.environ.get("ES_TRACE", "0") == "1",
            )
            break
        except Exception as exc:  # noqa: BLE001 - device flake, retry
            last_exc = exc
            import time as _time
            _time.sleep(2.0)
    else:
        raise last_exc
    kernel.last_results = res
    out = np.stack(
        [sum(res.results[b][f"out{q}"][:T_SLOTS].astype(np.float32)
             for q in range(NCHAINS))
         for b in range(B)],
        axis=0)
    return out.astype(np.float32)
